# revision 1
# baseline (speedup 1.0000x reference)
"""GCN (3-layer + linear head) on 8 Trainium2 NeuronCores.

Strategy (graph/data parallel, per the sharding hint):
  - Nodes are partitioned across the 8 cores by contiguous range (6250 each,
    padded to 6272 = 49*128). Each core owns the aggregation (scatter targets)
    for its node range; the small weight matrices are replicated.
  - Per layer: each core computes h' = (x_local @ W) * dinv_local, then two
    AllGathers replicate the full scaled feature table (split in half so the
    second collective overlaps with gathers against the first half, and so
    table row ids fit the int16 index format of the gather engine); each core
    then gathers rows for the sources of its incoming edges with batched
    dma_gather ops (2048 rows/op across 4 SWDGE queues).
  - The scatter-add (segment sum by destination) is done on the TensorEngine:
    edges are bucketed by destination tile (128 nodes) and padded to 16
    chunks of 128 edges (8 per table half); a host-built 0/1 selection matrix
    per chunk turns segment-sum into sel^T @ gathered_rows accumulated in
    PSUM (low half staged through SBUF while the high half's collective is
    still in flight).
  - Symmetric deg^-1/2 normalization is applied as a pre-scale of the table
    (source factor) and a post-scale of the aggregate (destination factor).

Host-side work is limited to index preprocessing of edge_index (bucketing,
padding, one-hot/selection layout) and data layout; all per-feature numeric
work (matmuls, gathers, aggregation, scaling, relu, head) runs on device.
"""

import os
import numpy as np

import concourse.bass as bass
import concourse.tile as tile
from concourse import bacc, mybir
from concourse.bass_utils import run_bass_kernel_spmd
from concourse.masks import make_identity

# ---- problem constants (hardcoded per contract) ----
N = 50000
E = 600000
D = 128
P = 128
NCORES = 8
NPC = N // NCORES            # 6250 nodes per core
NTILE = 49                   # ceil(6250/128)
NLOC = NTILE * P             # 6272 padded local nodes
HLOC = NLOC // 2             # 3136 rows per table half
HTAB = NCORES * HLOC         # 25088 rows per gathered half-table
CH = 8                       # chunks (128 edges) per dst tile per half
CT = 2 * CH                  # 16 chunks per dst tile total
GRP = 4                      # dst tiles per gather op
BB = GRP * CH * P            # 4096-row gather batches
NPAIR = (NTILE + GRP - 1) // GRP   # 13 (last covers a single tile)

f32 = mybir.dt.float32
i16 = mybir.dt.int16
i32 = mybir.dt.int32

TABLE_BF16 = True   # bf16 feature table + selection matmuls (f32 fallback)

_CACHE = {}


def _install_ntff_hook():
    """Best-effort NTFF profiling hook (used only when BASS_GCN_TRACE=1)."""
    try:
        import sys, types
        from trn_agent_boot.trn_boot import _ntff_profile_via_ctypes
        hook = _ntff_profile_via_ctypes('/opt/axon/libaxon_pjrt.so')
        if hook is None:
            return
        mod = types.ModuleType("antenv.axon_hooks")
        mod.get_axon_ntff_profile_hook = lambda: hook
        import antenv
        sys.modules['antenv.axon_hooks'] = mod
        antenv.axon_hooks = mod
    except Exception:
        pass


def _build_program():
    """Emit the per-core SPMD Bass program (same NEFF on all 8 cores)."""
    nc = bacc.Bacc(None, target_bir_lowering=False, num_swdge_queues=4)

    # ---- I/O ----
    xT_in = nc.dram_tensor("xT", [P, NLOC], f32, kind="ExternalInput")
    dinv_in = nc.dram_tensor("dinv", [P, NTILE], f32, kind="ExternalInput")
    gidx_lo_in = nc.dram_tensor("gidx_lo", [P, NTILE * CH * P // 16], i16,
                                kind="ExternalInput")
    gidx_hi_in = nc.dram_tensor("gidx_hi", [P, NTILE * CH * P // 16], i16,
                                kind="ExternalInput")
    f8 = mybir.dt.float8e4
    osel_lo_in = nc.dram_tensor("osel_lo", [NTILE * P, CH * P], f8, kind="ExternalInput")
    osel_hi_in = nc.dram_tensor("osel_hi", [NTILE * P, CH * P], f8, kind="ExternalInput")
    w_in = {}
    for wn in ("W1", "R1", "W2", "R2", "W3", "Wh"):
        w_in[wn] = nc.dram_tensor(wn, [D, D], f32, kind="ExternalInput")
    bh_in = nc.dram_tensor("bh", [1, D], f32, kind="ExternalInput")
    y_out = nc.dram_tensor("y", [NLOC, D], f32, kind="ExternalOutput")

    # ---- internal DRAM: per-layer collective buffers ----
    tdt = mybir.dt.bfloat16 if TABLE_BF16 else f32
    srcs, dlo, dhi = [], [], []
    for l in range(3):
        srcs.append(nc.dram_tensor(f"agsrc{l}", [NLOC, D], tdt))
        dlo.append(nc.dram_tensor(f"aglo{l}", [HTAB, D], tdt, addr_space="Shared"))
        dhi.append(nc.dram_tensor(f"aghi{l}", [HTAB, D], tdt, addr_space="Shared"))

    with tile.TileContext(nc) as tc:
        with tc.tile_pool(name="const", bufs=1) as cp, \
             tc.tile_pool(name="state", bufs=1) as sp, \
             tc.tile_pool(name="glo", bufs=3) as glp, \
             tc.tile_pool(name="ghi", bufs=3) as ghp, \
             tc.tile_pool(name="opool", bufs=3) as op, \
             tc.tile_pool(name="opoolh", bufs=3) as oph, \
             tc.tile_pool(name="work", bufs=4) as wp, \
             tc.tile_pool(name="psA", bufs=2, space="PSUM") as psA, \
             tc.tile_pool(name="psB", bufs=2, space="PSUM") as psB, \
             tc.tile_pool(name="psT", bufs=2, space="PSUM") as psT, \
             tc.tile_pool(name="psR", bufs=2, space="PSUM") as psR:

            # ---- resident state ----
            ident = cp.tile([P, P], f32)
            make_identity(nc, ident[:])
            ones1 = cp.tile([1, P], f32)
            nc.vector.memset(ones1[:], 1.0)
            W = {}
            for wn in ("W1", "R1", "W2", "R2", "W3", "Wh"):
                W[wn] = cp.tile([D, D], f32, name=wn + "_t", tag=wn)
                nc.sync.dma_start(out=W[wn][:], in_=w_in[wn][:])
            bh_t = cp.tile([1, D], f32)
            nc.sync.dma_start(out=bh_t[:], in_=bh_in[:])
            dinv_t = cp.tile([P, NTILE], f32)
            nc.sync.dma_start(out=dinv_t[:], in_=dinv_in[:])
            gidx_lo = cp.tile([P, NTILE * CH * P // 16], i16)
            nc.sync.dma_start(out=gidx_lo[:], in_=gidx_lo_in[:])
            gidx_hi = cp.tile([P, NTILE * CH * P // 16], i16)
            nc.sync.dma_start(out=gidx_hi[:], in_=gidx_hi_in[:])

            xT = sp.tile([P, NLOC], f32)      # resident x^T (feat-major)
            nc.sync.dma_start(out=xT[:], in_=xT_in[:])
            rres = sp.tile([P, NLOC], f32)    # residual tiles (node-major)
            aggL = sp.tile([P, NLOC], f32)    # low-half partial aggregates

            def ts(t):
                return slice(t * P, (t + 1) * P)

            def pair_tiles(j):
                return [t for t in range(GRP * j, GRP * (j + 1)) if t < NTILE]

            layers = [
                ("W1", "R1", True),
                ("W2", "R2", True),
                ("W3", None, False),
            ]

            for l, (wn, rn, act) in enumerate(layers):
                # ---- phase A: h' = (x @ W) * dinv -> agsrc; r = x @ R in SBUF
                for t in range(NTILE):
                    pA = psA.tile([P, D], f32, space="PSUM", tag="pA")
                    nc.tensor.matmul(out=pA[:], lhsT=xT[:, ts(t)], rhs=W[wn][:],
                                     start=True, stop=True)
                    hp = wp.tile([P, D], tdt, tag="hp")
                    nc.vector.tensor_tensor(
                        out=hp[:], in0=pA[:],
                        in1=dinv_t[:, t:t + 1].to_broadcast([P, D]),
                        op=mybir.AluOpType.mult)
                    nc.scalar.dma_start(out=srcs[l][ts(t), :], in_=hp[:])
                    if rn is not None:
                        pR = psR.tile([P, D], f32, space="PSUM", tag="pR")
                        nc.tensor.matmul(out=pR[:], lhsT=xT[:, ts(t)], rhs=W[rn][:],
                                         start=True, stop=True)
                        nc.vector.tensor_copy(out=rres[:, ts(t)], in_=pR[:])

                # ---- collectives: replicate the two table halves
                nc.gpsimd.collective_compute(
                    "AllGather", mybir.AluOpType.bypass,
                    replica_groups=[list(range(NCORES))],
                    ins=[srcs[l][0:HLOC, :]], outs=[dlo[l][:]],
                )
                nc.gpsimd.collective_compute(
                    "AllGather", mybir.AluOpType.bypass,
                    replica_groups=[list(range(NCORES))],
                    ins=[srcs[l][HLOC:NLOC, :]], outs=[dhi[l][:]],
                )

                # ---- low half: gather + partial aggregate into SBUF
                for j in range(NPAIR):
                    tls = pair_tiles(j)
                    nb = len(tls) * CH
                    g = glp.tile([P, GRP * CH, P], tdt, tag="glo")
                    nc.gpsimd.dma_gather(
                        out_ap=g[:, :nb, :], in_ap=dlo[l][:],
                        idxs_ap=gidx_lo[:, j * (BB // 16):
                                        j * (BB // 16) + nb * P // 16],
                        num_idxs=nb * P, num_idxs_reg=nb * P, elem_size=D,
                        queue_num=j % 4, single_packet=False)
                    for ti, t in enumerate(tls):
                        o8 = op.tile([P, CH * P], mybir.dt.float8e4, tag="o8")
                        nc.sync.dma_start(out=o8[:], in_=osel_lo_in[ts(t), :])
                        o = op.tile([P, CH * P], tdt, tag="o")
                        nc.scalar.copy(out=o[:], in_=o8[:])
                        pB = psB.tile([P, D], f32, space="PSUM", tag="pB")
                        for k in range(CH):
                            nc.tensor.matmul(
                                out=pB[:],
                                lhsT=o[:, k * P:(k + 1) * P],
                                rhs=g[:, ti * CH + k, :],
                                start=(k == 0), stop=(k == CH - 1))
                        nc.vector.tensor_copy(out=aggL[:, ts(t)], in_=pB[:])

                # ---- high half: gather + finish aggregate + epilogue
                for j in range(NPAIR):
                    tls = pair_tiles(j)
                    nb = len(tls) * CH
                    g = ghp.tile([P, GRP * CH, P], tdt, tag="ghi")
                    nc.gpsimd.dma_gather(
                        out_ap=g[:, :nb, :], in_ap=dhi[l][:],
                        idxs_ap=gidx_hi[:, j * (BB // 16):
                                        j * (BB // 16) + nb * P // 16],
                        num_idxs=nb * P, num_idxs_reg=nb * P, elem_size=D,
                        queue_num=j % 4, single_packet=False)
                    for ti, t in enumerate(tls):
                        o8 = oph.tile([P, CH * P], mybir.dt.float8e4, tag="oh8")
                        nc.sync.dma_start(out=o8[:], in_=osel_hi_in[ts(t), :])
                        o = oph.tile([P, CH * P], tdt, tag="oh")
                        nc.scalar.copy(out=o[:], in_=o8[:])
                        pB = psB.tile([P, D], f32, space="PSUM", tag="pB")
                        for k in range(CH):
                            nc.tensor.matmul(
                                out=pB[:],
                                lhsT=o[:, k * P:(k + 1) * P],
                                rhs=g[:, ti * CH + k, :],
                                start=(k == 0), stop=(k == CH - 1))
                        xn = wp.tile([P, D], f32, tag="xn")
                        nc.vector.tensor_add(out=xn[:], in0=pB[:],
                                             in1=aggL[:, ts(t)])
                        nc.vector.tensor_tensor(
                            out=xn[:], in0=xn[:],
                            in1=dinv_t[:, t:t + 1].to_broadcast([P, D]),
                            op=mybir.AluOpType.mult)
                        if rn is not None:
                            nc.vector.tensor_add(out=xn[:], in0=xn[:],
                                                 in1=rres[:, ts(t)])
                        if act:
                            nc.vector.tensor_scalar_max(out=xn[:], in0=xn[:],
                                                        scalar1=0.0)
                        pT = psT.tile([P, P], f32, space="PSUM", tag="pT")
                        nc.tensor.transpose(out=pT[:], in_=xn[:], identity=ident[:])
                        nc.vector.tensor_copy(out=xT[:, ts(t)], in_=pT[:])

                        if l == 2:
                            pH = psA.tile([P, D], f32, space="PSUM", tag="pA")
                            nc.tensor.matmul(out=pH[:], lhsT=xT[:, ts(t)],
                                             rhs=W["Wh"][:], start=True, stop=False)
                            nc.tensor.matmul(out=pH[:], lhsT=ones1[:], rhs=bh_t[:],
                                             start=False, stop=True)
                            yt = wp.tile([P, D], f32, tag="yt")
                            nc.vector.tensor_copy(out=yt[:], in_=pH[:])
                            nc.scalar.dma_start(out=y_out[ts(t), :], in_=yt[:])

    nc.compile()
    return nc


def _pack_gidx(slots):
    """slots: int array (multiple of 16) in op order -> [128, cols] int16.

    dma_gather reads index i of an op at partition i%16, column i//16,
    replicated across the 8 q7 cores (i.e. to all 128 partitions).
    """
    total = len(slots)
    cols = total // 16
    arr = np.empty((16, cols), np.int16)
    arr[np.arange(total) % 16, np.arange(total) // 16] = slots
    return np.ascontiguousarray(np.tile(arr, (8, 1)))


def _preprocess(x, edge_index):
    """Bucket/pad edges; build per-core input maps."""
    x = np.ascontiguousarray(np.asarray(x, dtype=np.float32))
    ei = np.asarray(edge_index)
    row = np.concatenate([ei[0].astype(np.int64), np.arange(N, dtype=np.int64)])
    col = np.concatenate([ei[1].astype(np.int64), np.arange(N, dtype=np.int64)])

    deg = np.bincount(col, minlength=N).astype(np.float32)
    dinv = deg ** -0.5  # deg >= 1 (self loops)

    # source -> (half, half-table row)
    src_core = row // NPC
    src_i = row - src_core * NPC
    src_hi = src_i >= HLOC
    src_tab = src_core * HLOC + np.where(src_hi, src_i - HLOC, src_i)

    dst_core = col // NPC
    dst_loc = col - dst_core * NPC

    ins = []
    for c in range(NCORES):
        m = {}
        sel = dst_core == c
        s_tab = src_tab[sel].astype(np.int64)
        s_hi = src_hi[sel]
        d_loc = dst_loc[sel].astype(np.int64)

        tile_id = d_loc >> 7
        d_in = d_loc & 127

        osel = np.zeros((NTILE, P, CT, P), np.float32)  # [tile, lane, chunk, dst]
        slots_lo = np.zeros((NTILE, CH * P), np.int64)  # pad -> row 0
        slots_hi = np.zeros((NTILE, CH * P), np.int64)

        for half, slots in ((0, slots_lo), (1, slots_hi)):
            hsel = s_hi == bool(half)
            st = s_tab[hsel]
            td = tile_id[hsel]
            dd = d_in[hsel]
            order = np.argsort(td, kind="stable")
            st, td, dd = st[order], td[order], dd[order]
            counts = np.bincount(td, minlength=NTILE)
            if counts.max() > CH * P:
                raise ValueError(
                    f"half-tile with {counts.max()} edges exceeds {CH*P}")
            starts = np.zeros(NTILE + 1, np.int64)
            np.cumsum(counts, out=starts[1:])
            pos = np.arange(len(td)) - starts[td]
            slots[td, pos] = st
            ch = (pos >> 7) + half * CH
            lane = pos & 127
            osel[td, lane, ch, dd] = 1.0

        osel = osel.reshape(NTILE * P, CT * P)
        f8np = mybir.dt.np(mybir.dt.float8e4)
        m["osel_lo"] = np.ascontiguousarray(osel[:, :CH * P]).astype(f8np)
        m["osel_hi"] = np.ascontiguousarray(osel[:, CH * P:]).astype(f8np)
        m["gidx_lo"] = _pack_gidx(slots_lo.reshape(-1))
        m["gidx_hi"] = _pack_gidx(slots_hi.reshape(-1))

        xl = np.zeros((NLOC, D), np.float32)
        xl[:NPC] = x[c * NPC:(c + 1) * NPC]
        m["xT"] = np.ascontiguousarray(xl.T)

        dv = np.zeros(NLOC, np.float32)
        dv[:NPC] = dinv[c * NPC:(c + 1) * NPC]
        m["dinv"] = np.ascontiguousarray(dv.reshape(NTILE, P).T)  # [lane, tile]
        ins.append(m)
    return ins


LAST_EXEC_NS = None


def kernel(x, edge_index, W1, R1, W2, R2, W3, Wh, bh):
    global LAST_EXEC_NS
    trace = os.environ.get("BASS_GCN_TRACE", "0") == "1"
    if trace:
        _install_ntff_hook()

    if "nc" not in _CACHE:
        _CACHE["nc"] = _build_program()
    nc = _CACHE["nc"]

    ins = _preprocess(x, edge_index)
    wmap = {"W1": W1, "R1": R1, "W2": W2, "R2": R2, "W3": W3, "Wh": Wh}
    for m in ins:
        for k, v in wmap.items():
            m[k] = np.ascontiguousarray(np.asarray(v, dtype=np.float32))
        m["bh"] = np.ascontiguousarray(
            np.asarray(bh, dtype=np.float32).reshape(1, D))

    res = run_bass_kernel_spmd(
        nc, ins, core_ids=list(range(NCORES)), trace=trace)
    LAST_EXEC_NS = res.exec_time_ns

    out = np.empty((N, D), np.float32)
    for c in range(NCORES):
        out[c * NPC:(c + 1) * NPC] = res.results[c]["y"][:NPC]
    return out



# revision 10
# speedup vs baseline: 1.9605x; 1.9605x over previous
"""GCN (3-layer + linear head) on 8 Trainium2 NeuronCores.

Strategy (graph/data parallel, per the sharding hint):
  - Nodes are partitioned across the 8 cores by contiguous range (6250 each,
    padded to 6272 = 49*128). Each core owns the aggregation (scatter targets)
    for its node range; the small weight matrices are replicated.
  - Per layer: each core computes h' = (x_local @ W) * dinv_local, then ONE
    AllGather replicates the full scaled feature table (50176 rows bf16).
    Table rows are addressed through a [25088, 256] "pair view" (two nodes per
    512-byte row) so gather indices fit int16; edges are bucketed by the
    parity of their source's table row and gathered with a 256B element and a
    512B stride (even srcs read cols 0:128 of the pair view, odd srcs cols
    128:256).
  - Gathers are issued as one dma_gather per (dst-tile, parity) bucket,
    striped round-robin over all 4 SWDGE queues: descriptor generation for
    different queues runs concurrently on different Q7 core pairs, which is
    the main throughput lever (a single queue serializes at ~8ns/row).
    Bucket sizes are compiled from the observed per-core edge counts (max
    across cores); each core pads its bucket tail with -1 indices, which the
    gather ucode trims so padding costs no descriptor work.
  - The scatter-add (segment sum by destination) runs on the TensorEngine:
    a host-built one-hot fp8 selection matrix per 128-edge chunk turns
    segment-sum into sel^T @ gathered_rows accumulated in PSUM (fp8 lhsT
    against bf16 rhs directly; no cast).
  - Self-loop edges never enter the gather: their contribution
    h'[i]*dinv[i] is added in the epilogue from the resident h' tile.
  - Symmetric deg^-1/2 normalization is applied as a pre-scale of the table
    (source factor) and a post-scale of the aggregate (destination factor).

Host-side work is limited to index preprocessing of edge_index (bucketing,
padding, one-hot/selection layout) and data layout; all per-feature numeric
work (matmuls, gathers, aggregation, scaling, relu, head) runs on device.
"""

import os
import numpy as np

import concourse.bass as bass
import concourse.tile as tile
from concourse import bacc, mybir
from concourse.bass_utils import run_bass_kernel_spmd
from concourse.masks import make_identity

# ---- problem constants (hardcoded per contract) ----
N = 50000
E = 600000
D = 128
P = 128
NCORES = 8
NPC = N // NCORES            # 6250 nodes per core
NTILE = 49                   # ceil(6250/128)
NLOC = NTILE * P             # 6272 padded local nodes
NPAIR_ROWS = NCORES * NLOC // 2   # 25088 pair rows in the gathered table
MCAP = 8                     # gather-buffer capacity in 128-edge chunks
NQ = 4                       # SWDGE queues (max supported by ucode)
GBUFS = 6                    # gather buffers per parity stream

f32 = mybir.dt.float32
bf16 = mybir.dt.bfloat16
f8 = mybir.dt.float8e4
i16 = mybir.dt.int16

_CACHE = {}


def _install_ntff_hook():
    """Best-effort NTFF profiling hook (used only when BASS_GCN_TRACE=1)."""
    try:
        import sys, types
        from trn_agent_boot.trn_boot import _ntff_profile_via_ctypes
        hook = _ntff_profile_via_ctypes('/opt/axon/libaxon_pjrt.so')
        if hook is None:
            return
        mod = types.ModuleType("antenv.axon_hooks")
        mod.get_axon_ntff_profile_hook = lambda: hook
        import antenv
        sys.modules['antenv.axon_hooks'] = mod
        antenv.axon_hooks = mod
    except Exception:
        pass


def _build_program(m):
    """Emit the per-core SPMD Bass program.

    m: int array [NTILE, 2] — chunks per (dst tile, src parity) bucket
       (max over cores, so the compiled structure is SPMD-uniform).
    """
    mtot = [int(m[:, s].sum()) for s in (0, 1)]       # chunks per stream
    cum = np.zeros((NTILE + 1, 2), np.int64)
    cum[1:] = np.cumsum(m, axis=0)

    nc = bacc.Bacc(None, target_bir_lowering=False, num_swdge_queues=NQ)

    # ---- I/O ----
    xT_in = nc.dram_tensor("xT", [P, NLOC], f32, kind="ExternalInput")
    dinv_in = nc.dram_tensor("dinv", [P, NTILE], f32, kind="ExternalInput")
    gidx_in = [nc.dram_tensor(f"gidx{s}", [P, mtot[s] * P // 16], i16,
                              kind="ExternalInput") for s in (0, 1)]
    osel_in = [nc.dram_tensor(f"osel{s}", [P, mtot[s] * P], f8,
                              kind="ExternalInput") for s in (0, 1)]
    w_in = {}
    for wn in ("W1", "R1", "W2", "R2", "W3", "Wh"):
        w_in[wn] = nc.dram_tensor(wn, [D, D], f32, kind="ExternalInput")
    bh_in = nc.dram_tensor("bh", [1, D], f32, kind="ExternalInput")
    y_out = nc.dram_tensor("y", [NLOC, D], f32, kind="ExternalOutput")

    # ---- internal DRAM: per-layer collective buffers ----
    srcs = [nc.dram_tensor(f"agsrc{l}", [NLOC, D], bf16) for l in range(3)]
    # table in pair view: row p holds nodes 2p (cols 0:128) and 2p+1 (128:256)
    tabs = [nc.dram_tensor(f"agtab{l}", [NPAIR_ROWS, 2 * D], bf16,
                           addr_space="Shared") for l in range(3)]

    with tile.TileContext(nc) as tc:
        with tc.tile_pool(name="const", bufs=1) as cp, \
             tc.tile_pool(name="state", bufs=1) as sp, \
             tc.tile_pool(name="ge", bufs=GBUFS) as gep, \
             tc.tile_pool(name="go", bufs=GBUFS) as gop, \
             tc.tile_pool(name="work", bufs=4) as wp, \
             tc.tile_pool(name="psA", bufs=2, space="PSUM") as psA, \
             tc.tile_pool(name="psR", bufs=2, space="PSUM") as psR, \
             tc.tile_pool(name="psB", bufs=2, space="PSUM") as psB, \
             tc.tile_pool(name="psT", bufs=2, space="PSUM") as psT:

            # ---- resident state ----
            ident = cp.tile([P, P], f32)
            make_identity(nc, ident[:])
            ones1 = cp.tile([1, P], f32)
            nc.vector.memset(ones1[:], 1.0)
            W = {}
            for wn in ("W1", "R1", "W2", "R2", "W3", "Wh"):
                W[wn] = cp.tile([D, D], f32, name=wn + "_t", tag=wn)
                nc.sync.dma_start(out=W[wn][:], in_=w_in[wn][:])
            bh_t = cp.tile([1, D], f32)
            nc.sync.dma_start(out=bh_t[:], in_=bh_in[:])
            dinv_t = cp.tile([P, NTILE], f32)
            nc.sync.dma_start(out=dinv_t[:], in_=dinv_in[:])
            gidx = []
            osel = []
            for s in (0, 1):
                gt = cp.tile([P, mtot[s] * P // 16], i16, tag=f"gidx{s}")
                nc.sync.dma_start(out=gt[:], in_=gidx_in[s][:])
                gidx.append(gt)
                ot = cp.tile([P, mtot[s] * P], f8, tag=f"osel{s}")
                nc.sync.dma_start(out=ot[:], in_=osel_in[s][:])
                osel.append(ot)

            xT = sp.tile([P, NLOC], f32)      # resident x^T (feat-major)
            nc.sync.dma_start(out=xT[:], in_=xT_in[:])
            hself = sp.tile([P, NLOC], bf16)  # h' = (x@W)*dinv, node-major
            rres = sp.tile([P, NLOC], bf16)   # residual x@R, node-major

            def ts(t):
                return slice(t * P, (t + 1) * P)

            layers = [
                ("W1", "R1", True),
                ("W2", "R2", True),
                ("W3", None, False),
            ]

            # global gather counter: queue = q % NQ must stay congruent with
            # Tile's DMASW sem-lane rotation (8 lanes), so never reset this
            # per layer.
            q = 0

            # initialize gather buffers (stale-NaN guard for skipped slots)
            gbufs = {}
            for s, pool in ((0, gep), (1, gop)):
                for _ in range(GBUFS):
                    g = pool.tile([P, MCAP, D], bf16, tag=f"g{s}")
                    nc.vector.memset(g[:], 0.0)

            for l, (wn, rn, act) in enumerate(layers):
                # ---- phase A: h' = (x @ W) * dinv -> hself + agsrc;
                #      r = x @ R kept in SBUF
                for t in range(NTILE):
                    pA = psA.tile([P, D], f32, space="PSUM", tag="pA")
                    nc.tensor.matmul(out=pA[:], lhsT=xT[:, ts(t)], rhs=W[wn][:],
                                     start=True, stop=True)
                    nc.vector.tensor_tensor(
                        out=hself[:, ts(t)], in0=pA[:],
                        in1=dinv_t[:, t:t + 1].to_broadcast([P, D]),
                        op=mybir.AluOpType.mult)
                    nc.scalar.dma_start(out=srcs[l][ts(t), :],
                                        in_=hself[:, ts(t)])
                    if rn is not None:
                        pR = psR.tile([P, D], f32, space="PSUM", tag="pR")
                        nc.tensor.matmul(out=pR[:], lhsT=xT[:, ts(t)],
                                         rhs=W[rn][:], start=True, stop=True)
                        nc.vector.tensor_copy(out=rres[:, ts(t)], in_=pR[:])

                # ---- collective: replicate the full scaled table
                nc.gpsimd.collective_compute(
                    "AllGather", mybir.AluOpType.bypass,
                    replica_groups=[list(range(NCORES))],
                    ins=[srcs[l][:]], outs=[tabs[l][:]],
                )

                # ---- gathers: one op per (tile, parity) bucket, 4 queues
                for t in range(NTILE):
                    for s in (0, 1):
                        mk = int(m[t, s])
                        pool = gep if s == 0 else gop
                        g = pool.tile([P, MCAP, D], bf16, tag=f"g{s}")
                        off = int(cum[t, s])
                        nc.gpsimd.dma_gather(
                            out_ap=g[:, :mk, :],
                            in_ap=tabs[l][:, s * D:(s + 1) * D],
                            idxs_ap=gidx[s][:, off * 8:(off + mk) * 8],
                            num_idxs=mk * P, num_idxs_reg=mk * P,
                            elem_size=D, elem_step=2 * D,
                            queue_num=q % NQ, single_packet=False)
                        q += 1
                        gbufs[(s, t)] = g

                # ---- aggregation + epilogue per tile
                for t in range(NTILE):
                    pB = psB.tile([P, D], f32, space="PSUM", tag="pB")
                    nchunks = int(m[t, 0] + m[t, 1])
                    k = 0
                    for s in (0, 1):
                        g = gbufs[(s, t)]
                        for c in range(int(m[t, s])):
                            col = (int(cum[t, s]) + c) * P
                            nc.tensor.matmul(
                                out=pB[:],
                                lhsT=osel[s][:, col:col + P],
                                rhs=g[:, c, :],
                                start=(k == 0), stop=(k == nchunks - 1))
                            k += 1
                    xn = wp.tile([P, D], f32, tag="xn")
                    # add self-loop contribution, then dst-side scale
                    nc.vector.tensor_add(out=xn[:], in0=pB[:],
                                         in1=hself[:, ts(t)])
                    nc.vector.tensor_tensor(
                        out=xn[:], in0=xn[:],
                        in1=dinv_t[:, t:t + 1].to_broadcast([P, D]),
                        op=mybir.AluOpType.mult)
                    if rn is not None:
                        nc.vector.tensor_add(out=xn[:], in0=xn[:],
                                             in1=rres[:, ts(t)])
                    if act:
                        nc.vector.tensor_scalar_max(out=xn[:], in0=xn[:],
                                                    scalar1=0.0)
                    pT = psT.tile([P, P], f32, space="PSUM", tag="pT")
                    nc.tensor.transpose(out=pT[:], in_=xn[:],
                                        identity=ident[:])
                    nc.vector.tensor_copy(out=xT[:, ts(t)], in_=pT[:])
                    if l == 2:
                        pH = psA.tile([P, D], f32, space="PSUM", tag="pA")
                        nc.tensor.matmul(out=pH[:], lhsT=xT[:, ts(t)],
                                         rhs=W["Wh"][:], start=True, stop=False)
                        nc.tensor.matmul(out=pH[:], lhsT=ones1[:], rhs=bh_t[:],
                                         start=False, stop=True)
                        yt = wp.tile([P, D], f32, tag="yt")
                        nc.vector.tensor_copy(out=yt[:], in_=pH[:])
                        nc.scalar.dma_start(out=y_out[ts(t), :], in_=yt[:])

    nc.compile()
    return nc


def _pack_gidx(slots):
    """slots: int array (multiple of 16) in op order -> [128, cols] int16.

    dma_gather reads index i of an op at partition i%16, column i//16,
    replicated across the 8 q7 cores (i.e. to all 128 partitions).
    """
    total = len(slots)
    cols = total // 16
    arr = np.empty((16, cols), np.int16)
    arr[np.arange(total) % 16, np.arange(total) // 16] = slots
    return np.ascontiguousarray(np.tile(arr, (8, 1)))


def _preprocess(x, edge_index):
    """Bucket edges by (dst tile, src parity); build per-core input maps."""
    x = np.ascontiguousarray(np.asarray(x, dtype=np.float32))
    ei = np.asarray(edge_index)
    row = ei[0].astype(np.int64)
    col = ei[1].astype(np.int64)

    # degree including self loops
    deg = (np.bincount(col, minlength=N) + 1).astype(np.float32)
    dinv = deg ** -0.5

    src_core = row // NPC
    src_i = row - src_core * NPC
    trow = src_core * NLOC + src_i          # table row in the AG output
    pairi = trow >> 1
    par = (trow & 1).astype(np.int64)

    dst_core = col // NPC
    dst_loc = col - dst_core * NPC
    tile_id = dst_loc >> 7
    d_in = dst_loc & 127

    # per-core bucket counts -> SPMD-uniform chunk counts m[t, s]
    percore = []
    counts = np.zeros((NCORES, NTILE, 2), np.int64)
    for c in range(NCORES):
        selc = dst_core == c
        pc = {
            "pairi": pairi[selc], "par": par[selc],
            "tile": tile_id[selc], "d_in": d_in[selc],
        }
        np.add.at(counts[c], (pc["tile"], pc["par"]), 1)
        percore.append(pc)
    m = np.ceil(counts.max(axis=0) / P).astype(np.int64)  # [NTILE, 2]
    mtot = m.sum(axis=0)
    cum = np.zeros((NTILE + 1, 2), np.int64)
    cum[1:] = np.cumsum(m, axis=0)

    ins = []
    for c in range(NCORES):
        pc = percore[c]
        mp = {}
        for s in (0, 1):
            sel = pc["par"] == s
            tl = pc["tile"][sel]
            pi = pc["pairi"][sel]
            dd = pc["d_in"][sel]
            order = np.argsort(tl, kind="stable")
            tl, pi, dd = tl[order], pi[order], dd[order]
            starts = np.zeros(NTILE + 1, np.int64)
            np.cumsum(np.bincount(tl, minlength=NTILE), out=starts[1:])
            pos = np.arange(len(tl)) - starts[tl]   # position within bucket

            slots = np.zeros(int(mtot[s]) * P, np.int64)  # pad -> row 0
            gslot = cum[tl, s] * P + pos
            slots[gslot] = pi

            oselm = np.zeros((P, int(mtot[s]) * P), np.float32)
            ch = cum[tl, s] + (pos >> 7)
            lane = pos & 127
            oselm[lane, ch * P + dd] = 1.0
            mp[f"gidx{s}"] = _pack_gidx(slots)
            mp[f"osel{s}"] = np.ascontiguousarray(
                oselm.astype(mybir.dt.np(f8)))
        xl = np.zeros((NLOC, D), np.float32)
        xl[:NPC] = x[c * NPC:(c + 1) * NPC]
        mp["xT"] = np.ascontiguousarray(xl.T)
        dv = np.zeros(NLOC, np.float32)
        dv[:NPC] = dinv[c * NPC:(c + 1) * NPC]
        mp["dinv"] = np.ascontiguousarray(dv.reshape(NTILE, P).T)
        ins.append(mp)
    return ins, m


LAST_EXEC_NS = None


def kernel(x, edge_index, W1, R1, W2, R2, W3, Wh, bh):
    global LAST_EXEC_NS
    trace = os.environ.get("BASS_GCN_TRACE", "0") == "1"
    if trace:
        _install_ntff_hook()

    ins, m = _preprocess(x, edge_index)

    key = m.tobytes()
    if _CACHE.get("key") != key:
        _CACHE["nc"] = _build_program(m)
        _CACHE["key"] = key
    nc = _CACHE["nc"]

    wmap = {"W1": W1, "R1": R1, "W2": W2, "R2": R2, "W3": W3, "Wh": Wh}
    for mp in ins:
        for k, v in wmap.items():
            mp[k] = np.ascontiguousarray(np.asarray(v, dtype=np.float32))
        mp["bh"] = np.ascontiguousarray(
            np.asarray(bh, dtype=np.float32).reshape(1, D))

    res = run_bass_kernel_spmd(
        nc, ins, core_ids=list(range(NCORES)), trace=trace)
    LAST_EXEC_NS = res.exec_time_ns

    out = np.empty((N, D), np.float32)
    for c in range(NCORES):
        out[c * NPC:(c + 1) * NPC] = res.results[c]["y"][:NPC]
    return out
